# revision 8
# baseline (speedup 1.0000x reference)
"""Multi-head attention (QKV proj -> causal attention -> out proj) on 8 TRN2 cores.

Sharding: batch x head-group. Core c handles batch c//4, heads (c%4)*4 .. +4.
Each core:
  - projects its batch's q/k/v rows with its 256-column slice of Wq/Wk/Wv
    (keeping Q^T / K^T with head-dim on partitions, V natural with an extra
    ones-column so the attention-weight row sums fall out of the A@V matmul),
  - computes scores twice (transposed [k,q] for the A@V contraction, natural
    [q,k] for the p_attn output rows), exp via ScalarE without max-subtraction
    (scores are O(6), fp32 exp is safe), causal masking via affine_select,
    softmax normalization folded into the output side,
  - writes its 4 heads' p_attn shard and a partial (row-sharded Wo) output
    projection; host sums the partials and adds bo.

Matmuls run in float32r (TF32-like, 1 cycle/row vs 4 for fp32; measured
~1.6e-4 absmax rel error per matmul on N(0,1) data). Set MM_PRECISE=True
to rebuild with full-fp32 matmuls.

Returns (out, p_attn) matching the reference.
"""
import sys

if "/opt/trn_rl_repo" not in sys.path:
    sys.path.insert(0, "/opt/trn_rl_repo")

import numpy as np

B, S, D = 2, 2048, 1024
H, DH = 16, 64
HPC = 4          # heads per core
NCORES = 8
NQT = S // 128   # 16 query tiles of 128 rows
NI = S // 512    # 4 query super-tiles of 512 rows

TRACE = False
LAST_RESULT = None
WRITE_ZEROS = False
MM_PRECISE = False

_NC_CACHE = {}


def _build(causal, precise):
    import concourse.bacc as bacc
    import concourse.tile as tile
    from concourse import mybir
    from concourse.masks import make_identity
    from contextlib import ExitStack

    f32 = mybir.dt.float32
    mmdt = f32 if precise else mybir.dt.float32r
    Exp = mybir.ActivationFunctionType.Exp
    Ident = mybir.ActivationFunctionType.Identity
    is_ge = mybir.AluOpType.is_ge

    nc = bacc.Bacc("TRN2", target_bir_lowering=False, debug=False, num_devices=NCORES)

    xq = nc.dram_tensor("xq", [S, D], mmdt, kind="ExternalInput").ap()
    xk = nc.dram_tensor("xk", [S, D], mmdt, kind="ExternalInput").ap()
    xv = nc.dram_tensor("xv", [S, D], mmdt, kind="ExternalInput").ap()
    wq = nc.dram_tensor("wq", [D, HPC * DH], mmdt, kind="ExternalInput").ap()
    wk = nc.dram_tensor("wk", [D, HPC * DH], mmdt, kind="ExternalInput").ap()
    wv = nc.dram_tensor("wv", [D, HPC * DH], mmdt, kind="ExternalInput").ap()
    bq = nc.dram_tensor("bq", [HPC * DH], f32, kind="ExternalInput").ap()
    bk = nc.dram_tensor("bk", [HPC * DH], f32, kind="ExternalInput").ap()
    bv = nc.dram_tensor("bv", [HPC * DH], f32, kind="ExternalInput").ap()
    wo = nc.dram_tensor("wo", [HPC * DH, D], mmdt, kind="ExternalInput").ap()
    if not causal:
        mb = nc.dram_tensor("mb", [S, S], f32, kind="ExternalInput").ap()
        mbT = nc.dram_tensor("mbT", [S, S], f32, kind="ExternalInput").ap()
    p_out = nc.dram_tensor("p_out", [HPC, S, S], f32, kind="ExternalOutput").ap()
    o_part = nc.dram_tensor("o_part", [S, D], f32, kind="ExternalOutput").ap()

    with ExitStack() as ctx:
        tc = ctx.enter_context(tile.TileContext(nc))
        persist = ctx.enter_context(tc.tile_pool(name="persist", bufs=1))
        xpool = ctx.enter_context(tc.tile_pool(name="xpool", bufs=3))
        slabp = ctx.enter_context(tc.tile_pool(name="slabp", bufs=2))
        etp = ctx.enter_context(tc.tile_pool(name="etp", bufs=6))
        prp = ctx.enter_context(tc.tile_pool(name="prp", bufs=2))
        xanp = ctx.enter_context(tc.tile_pool(name="xanp", bufs=6))
        osbp = ctx.enter_context(tc.tile_pool(name="osbp", bufs=2))
        smallp = ctx.enter_context(tc.tile_pool(name="smallp", bufs=2))
        mbp = None
        if not causal:
            mbp = ctx.enter_context(tc.tile_pool(name="mbp", bufs=3))
        ps_a = ctx.enter_context(tc.tile_pool(name="ps_a", bufs=6, space="PSUM"))
        ps_x = ctx.enter_context(tc.tile_pool(name="ps_x", bufs=2, space="PSUM"))

        ident = persist.tile([128, 128], mmdt)
        identf = persist.tile([128, 128], f32)
        make_identity(nc, identf)
        nc.vector.tensor_copy(ident, identf)

        qT = persist.tile([128, 2, S], mmdt)     # [dh_local, chunk, s]
        kT = persist.tile([128, 2, S], mmdt)
        v_nat = persist.tile([128, NQT, HPC, DH + 1], mmdt)  # [s_loc, s_chunk, h, dh|1]
        onescol = persist.tile([128, 1], f32)
        nc.gpsimd.memset(onescol, 1.0)
        nc.vector.tensor_copy(
            v_nat[:, :, :, DH:DH + 1],
            onescol.to_broadcast([128, NQT, HPC, 1]))

        wq_sb = persist.tile([128, 8, HPC * DH], mmdt)
        wk_sb = persist.tile([128, 8, HPC * DH], mmdt)
        wv_sb = persist.tile([128, 8, HPC * DH], mmdt)
        nc.sync.dma_start(wq_sb, wq.rearrange("(c p) m -> p c m", p=128))
        nc.sync.dma_start(wk_sb, wk.rearrange("(c p) m -> p c m", p=128))
        nc.sync.dma_start(wv_sb, wv.rearrange("(c p) m -> p c m", p=128))
        wo_sb = persist.tile([64, HPC, D], mmdt)  # [dh, h, n]
        nc.sync.dma_start(wo_sb, wo.rearrange("(h p) n -> p h n", p=64))

        bq_sb = persist.tile([128, 2], f32)
        bk_sb = persist.tile([128, 2], f32)
        nc.sync.dma_start(bq_sb, bq.rearrange("(c p) -> p c", p=128))
        nc.sync.dma_start(bk_sb, bk.rearrange("(c p) -> p c", p=128))
        bv_sb = persist.tile([64, HPC], f32)   # [dh, h]
        nc.sync.dma_start(bv_sb, bv.rearrange("(h p) -> p h", p=64))

        ones_f = persist.tile([1, 64], f32)
        nc.gpsimd.memset(ones_f, 1.0)

        # additive causal masks. triG is a sliding window: slicing
        # triG[:, 384-o : 512] (width o+128) masks a transposed-scores diag
        # block whose k-tile starts o columns into the q-supertile.
        # triG[p,f] = -1e9 where (f-384) < p else 0.
        triG = persist.tile([128, 512], f32)
        nc.gpsimd.memset(triG, 0.0)
        nc.gpsimd.affine_select(out=triG, in_=triG, compare_op=is_ge,
                                fill=-1e9, base=-384, channel_multiplier=-1,
                                pattern=[[1, 512]])
        triN = persist.tile([128, 128], f32)
        nc.gpsimd.memset(triN, 0.0)
        nc.gpsimd.affine_select(out=triN, in_=triN, compare_op=is_ge,
                                fill=-1e9, base=0, channel_multiplier=1,
                                pattern=[[-1, 128]])

        zero_t = None
        if causal and WRITE_ZEROS:
            zero_t = persist.tile([128, S - 128], f32)
            nc.gpsimd.memset(zero_t, 0.0)

        # ---------------- phase A: projections ----------------
        for t_idx, (x_in, w_sb) in enumerate([(xq, wq_sb), (xk, wk_sb), (xv, wv_sb)]):
            for st in range(4):      # s-tiles of 512
                slab = slabp.tile([128, 8, 512], mmdt, tag="slab")
                for scn in range(4):  # s-chunks of 128
                    xt = xpool.tile([128, D], mmdt, tag="x")
                    s0 = st * 512 + scn * 128
                    nc.sync.dma_start(xt, x_in[s0:s0 + 128, :])
                    for dc in range(8):
                        trp = ps_a.tile([128, 128], mmdt, tag="bank")
                        nc.tensor.transpose(trp, xt[:, dc * 128:(dc + 1) * 128], ident)
                        nc.scalar.activation(
                            slab[:, dc, scn * 128:(scn + 1) * 128], trp, Ident)
                if t_idx < 2:
                    dest, b_sb = (qT, bq_sb) if t_idx == 0 else (kT, bk_sb)
                    for c in range(2):
                        pp = ps_a.tile([128, 512], f32, tag="bank")
                        for dc in range(8):
                            nc.tensor.matmul(
                                pp, w_sb[:, dc, c * 128:(c + 1) * 128],
                                slab[:, dc, :],
                                start=(dc == 0), stop=(dc == 7))
                        nc.scalar.activation(
                            dest[:, c, st * 512:(st + 1) * 512], pp, Ident,
                            bias=b_sb[:, c:c + 1])
                else:
                    for scn in range(4):
                        sc = st * 4 + scn
                        pp = ps_a.tile([128, 256], f32, tag="bank")
                        for dc in range(8):
                            nc.tensor.matmul(
                                pp, slab[:, dc, scn * 128:(scn + 1) * 128],
                                w_sb[:, dc, :],
                                start=(dc == 0), stop=(dc == 7))
                        nc.vector.tensor_copy(
                            v_nat[:, sc, :, 0:DH],
                            pp.rearrange("p (h d) -> p h d", h=HPC))

        # ---------------- phase B: attention ----------------
        for I in range(NI):
            xans = []          # per head [64, 512] normalized x_att^T (+bv)
            rns_list = []      # per head [128, 4] natural-layout recip
            for h in range(HPC):
                c, po = h // 2, (h % 2) * 64
                nj = 4 * I + 4 if causal else NQT
                xatt = ps_x.tile([DH + 1, 512], f32, tag="xatt")
                for j in range(nj):
                    scp = ps_a.tile([128, 512], f32, tag="bank")
                    nc.tensor.matmul(
                        scp,
                        kT[po:po + 64, c, j * 128:(j + 1) * 128],
                        qT[po:po + 64, c, I * 512:(I + 1) * 512],
                        start=True, stop=True)
                    if not causal:
                        mt = mbp.tile([128, 512], f32, tag="mbT")
                        nc.sync.dma_start(
                            mt, mbT[j * 128:(j + 1) * 128, I * 512:(I + 1) * 512])
                        nc.vector.tensor_add(scp, scp, mt)
                    if causal and j >= 4 * I:
                        o = (j - 4 * I) * 128
                        nc.vector.tensor_add(
                            scp[:, 0:o + 128], scp[:, 0:o + 128],
                            triG[:, 384 - o:512])
                    eT = etp.tile([128, 512], mmdt, tag="eT")
                    nc.scalar.activation(eT, scp, Exp, scale=0.125)
                    nc.tensor.matmul(
                        xatt, v_nat[:, j, h, :], eT,
                        start=(j == 0), stop=(j == nj - 1))
                # normalizers: bcs = 1/sums broadcast over 64 partitions,
                # rns = 1/sums in natural [q,1] layout
                sums = smallp.tile([1, 512], f32, tag="sums")
                nc.vector.tensor_copy(sums, xatt[DH:DH + 1, :])
                bcp = ps_a.tile([64, 512], f32, tag="bank")
                nc.tensor.matmul(bcp, ones_f, sums, start=True, stop=True)
                bcs = smallp.tile([64, 512], f32, tag="bcs")
                nc.vector.reciprocal(bcs, bcp)
                xan = xanp.tile([64, 512], mmdt, tag="xan")
                nc.vector.tensor_mul(xan, xatt[0:DH, :], bcs)
                nc.vector.tensor_scalar_add(xan, xan, bv_sb[:, h:h + 1])
                xans.append(xan)
                # natural-layout recip column per 128-query subtile
                rnp = ps_a.tile([128, 4], f32, tag="bank")
                for ii in range(4):
                    nc.tensor.matmul(
                        rnp[:, ii:ii + 1],
                        sums[0:1, ii * 128:(ii + 1) * 128],
                        ones_f[0:1, 0:1],
                        start=True, stop=True)
                rns = smallp.tile([128, 4], f32, tag="rns")
                nc.vector.reciprocal(rns, rnp)
                rns_list.append(rns)
            # natural side: p_attn rows
            for h in range(HPC):
                c, po = h // 2, (h % 2) * 64
                for ii in range(4):
                    gi = 4 * I + ii
                    nch = (gi // 4 + 1) if causal else 4
                    wlast = (gi % 4) * 128 + 128
                    prow = prp.tile([128, S], f32, tag="prow")
                    for cc in range(nch):
                        w = 512 if (not causal or cc < gi // 4) else wlast
                        nsp = ps_a.tile([128, 512], f32, tag="bank")
                        nc.tensor.matmul(
                            nsp[:, 0:w],
                            qT[po:po + 64, c, gi * 128:(gi + 1) * 128],
                            kT[po:po + 64, c, cc * 512:cc * 512 + w],
                            start=True, stop=True)
                        if not causal:
                            mt2 = mbp.tile([128, 512], f32, tag="mb")
                            nc.sync.dma_start(
                                mt2, mb[gi * 128:(gi + 1) * 128,
                                        cc * 512:(cc + 1) * 512])
                            nc.vector.tensor_add(nsp, nsp, mt2)
                        if causal and cc == nch - 1:
                            nc.vector.tensor_add(
                                nsp[:, w - 128:w], nsp[:, w - 128:w], triN)
                        nc.scalar.activation(
                            prow[:, cc * 512:cc * 512 + w], nsp[:, 0:w],
                            Exp, scale=0.125)
                    W = (gi + 1) * 128 if causal else S
                    nc.gpsimd.tensor_scalar_mul(
                        prow[:, 0:W], prow[:, 0:W],
                        rns_list[h][:, ii:ii + 1])
                    nc.sync.dma_start(
                        p_out[h, gi * 128:(gi + 1) * 128, 0:W], prow[:, 0:W])
                    if causal and WRITE_ZEROS and gi < NQT - 1:
                        nc.sync.dma_start(
                            p_out[h, gi * 128:(gi + 1) * 128, W:S],
                            zero_t[:, 0:S - W])
            # output projection for this 512-row block
            for ii in range(4):
                gi = 4 * I + ii
                osb = osbp.tile([128, D], f32, tag="osb")
                for nh in range(2):
                    opp = ps_a.tile([128, 512], f32, tag="bank")
                    for h in range(HPC):
                        nc.tensor.matmul(
                            opp,
                            xans[h][:, ii * 128:(ii + 1) * 128],
                            wo_sb[:, h, nh * 512:(nh + 1) * 512],
                            start=(h == 0), stop=(h == HPC - 1))
                    nc.vector.tensor_copy(osb[:, nh * 512:(nh + 1) * 512], opp)
                nc.sync.dma_start(o_part[gi * 128:(gi + 1) * 128, :], osb)

    nc.compile()
    return nc


def _get_nc(causal):
    key = (causal, MM_PRECISE, WRITE_ZEROS)
    if key not in _NC_CACHE:
        _NC_CACHE[key] = _build(causal, MM_PRECISE)
    return _NC_CACHE[key]


_TRIL = None


def _is_causal(mask):
    global _TRIL
    m = np.asarray(mask).reshape(S, S)
    if _TRIL is None:
        _TRIL = np.tril(np.ones((S, S), m.dtype))
    return np.array_equal(m, _TRIL)


def kernel(q, k, v, mask, Wq, bq, Wk, bk, Wv, bv, Wo, bo):
    from concourse import bass_utils
    global LAST_RESULT

    q = np.ascontiguousarray(np.asarray(q, np.float32))
    k = np.ascontiguousarray(np.asarray(k, np.float32))
    v = np.ascontiguousarray(np.asarray(v, np.float32))
    Wq, Wk, Wv, Wo = [np.asarray(t, np.float32) for t in (Wq, Wk, Wv, Wo)]
    bq, bk, bv, bo = [np.asarray(t, np.float32) for t in (bq, bk, bv, bo)]

    causal = _is_causal(mask)
    nc = _get_nc(causal)

    extra = {}
    if not causal:
        m = np.asarray(mask).reshape(S, S)
        mbias = np.where(m == 0, np.float32(-8e9), np.float32(0.0)).astype(np.float32)
        extra = {"mb": np.ascontiguousarray(mbias),
                 "mbT": np.ascontiguousarray(mbias.T)}

    in_maps = []
    for cidx in range(NCORES):
        b_, hg = divmod(cidx, 4)
        sl = slice(hg * HPC * DH, (hg + 1) * HPC * DH)
        in_maps.append({
            "xq": q[b_], "xk": k[b_], "xv": v[b_],
            "wq": np.ascontiguousarray(Wq[:, sl]),
            "wk": np.ascontiguousarray(Wk[:, sl]),
            "wv": np.ascontiguousarray(Wv[:, sl]),
            "bq": np.ascontiguousarray(bq[sl]),
            "bk": np.ascontiguousarray(bk[sl]),
            "bv": np.ascontiguousarray(bv[sl]),
            "wo": np.ascontiguousarray(Wo[sl, :]),
            **extra,
        })

    res = bass_utils.run_bass_kernel_spmd(
        nc, in_maps, core_ids=list(range(NCORES)), trace=TRACE)
    LAST_RESULT = res

    p_attn = np.empty((B, H, S, S), np.float32)
    out = np.zeros((B, S, D), np.float32)
    for cidx in range(NCORES):
        b_, hg = divmod(cidx, 4)
        r = res.results[cidx]
        p_attn[b_, hg * HPC:(hg + 1) * HPC] = r["p_out"]
        out[b_] += r["o_part"]
    out += bo
    return out, p_attn


# revision 9
# speedup vs baseline: 2.1426x; 2.1426x over previous
"""Multi-head attention (QKV proj -> causal attention -> out proj) on 8 TRN2 cores.

Sharding: batch x head-group. Core c handles batch c//4, heads (c%4)*4 .. +4.
Each core:
  - projects its batch's q/k/v rows with its 256-column slice of Wq/Wk/Wv
    (keeping Q^T / K^T with head-dim on partitions, V natural with an extra
    ones-column so the attention-weight row sums fall out of the A@V matmul),
  - computes scores twice (transposed [k,q] for the A@V contraction, natural
    [q,k] for the p_attn output rows), exp via ScalarE without max-subtraction
    (scores are O(6), fp32 exp is safe), causal masking via affine_select,
    softmax normalization folded into the output side,
  - writes its 4 heads' p_attn shard and a partial (row-sharded Wo) output
    projection; host sums the partials and adds bo.

Matmuls run in float32r (TF32-like, 1 cycle/row vs 4 for fp32; measured
~1.6e-4 absmax rel error per matmul on N(0,1) data). Set MM_PRECISE=True
to rebuild with full-fp32 matmuls.

Returns (out, p_attn) matching the reference.
"""
import sys

if "/opt/trn_rl_repo" not in sys.path:
    sys.path.insert(0, "/opt/trn_rl_repo")

import numpy as np

B, S, D = 2, 2048, 1024
H, DH = 16, 64
HPC = 4          # heads per core
NCORES = 8
NQT = S // 128   # 16 query tiles of 128 rows
NI = S // 512    # 4 query super-tiles of 512 rows

TRACE = False
LAST_RESULT = None
WRITE_ZEROS = False
MM_PRECISE = False

_NC_CACHE = {}


def _build(causal, precise):
    import concourse.bacc as bacc
    import concourse.tile as tile
    from concourse import mybir
    from concourse.masks import make_identity
    from contextlib import ExitStack

    f32 = mybir.dt.float32
    mmdt = f32 if precise else mybir.dt.float32r
    Exp = mybir.ActivationFunctionType.Exp
    Ident = mybir.ActivationFunctionType.Identity
    is_ge = mybir.AluOpType.is_ge

    nc = bacc.Bacc("TRN2", target_bir_lowering=False, debug=False, num_devices=NCORES)

    xq = nc.dram_tensor("xq", [S, D], mmdt, kind="ExternalInput").ap()
    xk = nc.dram_tensor("xk", [S, D], mmdt, kind="ExternalInput").ap()
    xv = nc.dram_tensor("xv", [S, D], mmdt, kind="ExternalInput").ap()
    wq = nc.dram_tensor("wq", [D, HPC * DH], mmdt, kind="ExternalInput").ap()
    wk = nc.dram_tensor("wk", [D, HPC * DH], mmdt, kind="ExternalInput").ap()
    wv = nc.dram_tensor("wv", [D, HPC * DH], mmdt, kind="ExternalInput").ap()
    bq = nc.dram_tensor("bq", [HPC * DH], f32, kind="ExternalInput").ap()
    bk = nc.dram_tensor("bk", [HPC * DH], f32, kind="ExternalInput").ap()
    bv = nc.dram_tensor("bv", [HPC * DH], f32, kind="ExternalInput").ap()
    wo = nc.dram_tensor("wo", [HPC * DH, D], mmdt, kind="ExternalInput").ap()
    if not causal:
        mb = nc.dram_tensor("mb", [S, S], f32, kind="ExternalInput").ap()
        mbT = nc.dram_tensor("mbT", [S, S], f32, kind="ExternalInput").ap()
    p_out = nc.dram_tensor("p_out", [HPC, S, S], f32, kind="ExternalOutput").ap()
    o_part = nc.dram_tensor("o_part", [S, D], f32, kind="ExternalOutput").ap()

    with ExitStack() as ctx:
        tc = ctx.enter_context(tile.TileContext(nc))
        persist = ctx.enter_context(tc.tile_pool(name="persist", bufs=1))
        xpool = ctx.enter_context(tc.tile_pool(name="xpool", bufs=3))
        slabp = ctx.enter_context(tc.tile_pool(name="slabp", bufs=2))
        etp = ctx.enter_context(tc.tile_pool(name="etp", bufs=6))
        prp = ctx.enter_context(tc.tile_pool(name="prp", bufs=2))
        xanp = ctx.enter_context(tc.tile_pool(name="xanp", bufs=6))
        osbp = ctx.enter_context(tc.tile_pool(name="osbp", bufs=2))
        smallp = ctx.enter_context(tc.tile_pool(name="smallp", bufs=2))
        mbp = None
        if not causal:
            mbp = ctx.enter_context(tc.tile_pool(name="mbp", bufs=3))
        ps_a = ctx.enter_context(tc.tile_pool(name="ps_a", bufs=6, space="PSUM"))
        ps_x = ctx.enter_context(tc.tile_pool(name="ps_x", bufs=2, space="PSUM"))

        ident = persist.tile([128, 128], mmdt)
        identf = persist.tile([128, 128], f32)
        make_identity(nc, identf)
        nc.vector.tensor_copy(ident, identf)

        qT = persist.tile([128, 2, S], mmdt)     # [dh_local, chunk, s]
        kT = persist.tile([128, 2, S], mmdt)
        v_nat = persist.tile([128, NQT, HPC, DH + 1], mmdt)  # [s_loc, s_chunk, h, dh|1]
        onescol = persist.tile([128, 1], f32)
        nc.gpsimd.memset(onescol, 1.0)
        nc.vector.tensor_copy(
            v_nat[:, :, :, DH:DH + 1],
            onescol.to_broadcast([128, NQT, HPC, 1]))

        wq_sb = persist.tile([128, 8, HPC * DH], mmdt)
        wk_sb = persist.tile([128, 8, HPC * DH], mmdt)
        wv_sb = persist.tile([128, 8, HPC * DH], mmdt)
        nc.sync.dma_start(wq_sb, wq.rearrange("(c p) m -> p c m", p=128))
        nc.sync.dma_start(wk_sb, wk.rearrange("(c p) m -> p c m", p=128))
        nc.sync.dma_start(wv_sb, wv.rearrange("(c p) m -> p c m", p=128))
        wo_sb = persist.tile([64, HPC, D], mmdt)  # [dh, h, n]
        nc.sync.dma_start(wo_sb, wo.rearrange("(h p) n -> p h n", p=64))

        bq_sb = persist.tile([128, 2], f32)
        bk_sb = persist.tile([128, 2], f32)
        nc.sync.dma_start(bq_sb, bq.rearrange("(c p) -> p c", p=128))
        nc.sync.dma_start(bk_sb, bk.rearrange("(c p) -> p c", p=128))
        bv_sb = persist.tile([64, HPC], f32)   # [dh, h]
        nc.sync.dma_start(bv_sb, bv.rearrange("(h p) -> p h", p=64))

        ones_f = persist.tile([1, 64], f32)
        nc.gpsimd.memset(ones_f, 1.0)

        # additive causal masks. triG is a sliding window: slicing
        # triG[:, 384-o : 512] (width o+128) masks a transposed-scores diag
        # block whose k-tile starts o columns into the q-supertile.
        # triG[p,f] = -1e9 where (f-384) < p else 0.
        triG = persist.tile([128, 512], f32)
        nc.gpsimd.memset(triG, 0.0)
        nc.gpsimd.affine_select(out=triG, in_=triG, compare_op=is_ge,
                                fill=-1e9, base=-384, channel_multiplier=-1,
                                pattern=[[1, 512]])
        triN = persist.tile([128, 128], f32)
        nc.gpsimd.memset(triN, 0.0)
        nc.gpsimd.affine_select(out=triN, in_=triN, compare_op=is_ge,
                                fill=-1e9, base=0, channel_multiplier=1,
                                pattern=[[-1, 128]])

        zero_t = None
        if causal and WRITE_ZEROS:
            zero_t = persist.tile([128, S - 128], f32)
            nc.gpsimd.memset(zero_t, 0.0)

        # ---------------- phase A: projections ----------------
        for t_idx, (x_in, w_sb) in enumerate([(xq, wq_sb), (xk, wk_sb), (xv, wv_sb)]):
            for st in range(4):      # s-tiles of 512
                slab = slabp.tile([128, 8, 512], mmdt, tag="slab")
                for scn in range(4):  # s-chunks of 128
                    xt = xpool.tile([128, D], mmdt, tag="x")
                    s0 = st * 512 + scn * 128
                    nc.sync.dma_start(xt, x_in[s0:s0 + 128, :])
                    for dc in range(8):
                        trp = ps_a.tile([128, 128], mmdt, tag="bank")
                        nc.tensor.transpose(trp, xt[:, dc * 128:(dc + 1) * 128], ident)
                        nc.scalar.activation(
                            slab[:, dc, scn * 128:(scn + 1) * 128], trp, Ident)
                if t_idx < 2:
                    dest, b_sb = (qT, bq_sb) if t_idx == 0 else (kT, bk_sb)
                    for c in range(2):
                        pp = ps_a.tile([128, 512], f32, tag="bank")
                        for dc in range(8):
                            nc.tensor.matmul(
                                pp, w_sb[:, dc, c * 128:(c + 1) * 128],
                                slab[:, dc, :],
                                start=(dc == 0), stop=(dc == 7))
                        nc.scalar.activation(
                            dest[:, c, st * 512:(st + 1) * 512], pp, Ident,
                            bias=b_sb[:, c:c + 1])
                else:
                    for scn in range(4):
                        sc = st * 4 + scn
                        pp = ps_a.tile([128, 256], f32, tag="bank")
                        for dc in range(8):
                            nc.tensor.matmul(
                                pp, slab[:, dc, scn * 128:(scn + 1) * 128],
                                w_sb[:, dc, :],
                                start=(dc == 0), stop=(dc == 7))
                        nc.vector.tensor_copy(
                            v_nat[:, sc, :, 0:DH],
                            pp.rearrange("p (h d) -> p h d", h=HPC))

        # ---------------- phase B: attention ----------------
        for I in range(NI):
            xans = []          # per head [64, 512] normalized x_att^T (+bv)
            rns_list = []      # per head [128, 4] natural-layout recip
            for h in range(HPC):
                c, po = h // 2, (h % 2) * 64
                nj = 4 * I + 4 if causal else NQT
                xatt = ps_x.tile([DH + 1, 512], f32, tag="xatt")
                for j in range(nj):
                    scp = ps_a.tile([128, 512], f32, tag="bank")
                    nc.tensor.matmul(
                        scp,
                        kT[po:po + 64, c, j * 128:(j + 1) * 128],
                        qT[po:po + 64, c, I * 512:(I + 1) * 512],
                        start=True, stop=True)
                    if not causal:
                        mt = mbp.tile([128, 512], f32, tag="mbT")
                        nc.sync.dma_start(
                            mt, mbT[j * 128:(j + 1) * 128, I * 512:(I + 1) * 512])
                        nc.vector.tensor_add(scp, scp, mt)
                    if causal and j >= 4 * I:
                        o = (j - 4 * I) * 128
                        nc.vector.tensor_add(
                            scp[:, 0:o + 128], scp[:, 0:o + 128],
                            triG[:, 384 - o:512])
                    eT = etp.tile([128, 512], mmdt, tag="eT")
                    nc.scalar.activation(eT, scp, Exp, scale=0.125)
                    nc.tensor.matmul(
                        xatt, v_nat[:, j, h, :], eT,
                        start=(j == 0), stop=(j == nj - 1))
                # normalizers: bcs = 1/sums broadcast over 64 partitions,
                # rns = 1/sums in natural [q,1] layout
                sums = smallp.tile([1, 512], f32, tag="sums")
                nc.vector.tensor_copy(sums, xatt[DH:DH + 1, :])
                bcp = ps_a.tile([64, 512], f32, tag="bank")
                nc.tensor.matmul(bcp, ones_f, sums, start=True, stop=True)
                bcs = smallp.tile([64, 512], f32, tag="bcs")
                nc.vector.reciprocal(bcs, bcp)
                xan = xanp.tile([64, 512], mmdt, tag="xan")
                nc.vector.tensor_mul(xan, xatt[0:DH, :], bcs)
                nc.vector.tensor_scalar_add(xan, xan, bv_sb[:, h:h + 1])
                xans.append(xan)
                # natural-layout recip column per 128-query subtile
                rnp = ps_a.tile([128, 4], f32, tag="bank")
                for ii in range(4):
                    nc.tensor.matmul(
                        rnp[:, ii:ii + 1],
                        sums[0:1, ii * 128:(ii + 1) * 128],
                        ones_f[0:1, 0:1],
                        start=True, stop=True)
                rns = smallp.tile([128, 4], f32, tag="rns")
                nc.vector.reciprocal(rns, rnp)
                rns_list.append(rns)
            # natural side: p_attn rows
            for h in range(HPC):
                c, po = h // 2, (h % 2) * 64
                for ii in range(4):
                    gi = 4 * I + ii
                    nch = (gi // 4 + 1) if causal else 4
                    wlast = (gi % 4) * 128 + 128
                    prow = prp.tile([128, S], f32, tag="prow")
                    for cc in range(nch):
                        w = 512 if (not causal or cc < gi // 4) else wlast
                        nsp = ps_a.tile([128, 512], f32, tag="bank")
                        nc.tensor.matmul(
                            nsp[:, 0:w],
                            qT[po:po + 64, c, gi * 128:(gi + 1) * 128],
                            kT[po:po + 64, c, cc * 512:cc * 512 + w],
                            start=True, stop=True)
                        if not causal:
                            mt2 = mbp.tile([128, 512], f32, tag="mb")
                            nc.sync.dma_start(
                                mt2, mb[gi * 128:(gi + 1) * 128,
                                        cc * 512:(cc + 1) * 512])
                            nc.vector.tensor_add(nsp, nsp, mt2)
                        if causal and cc == nch - 1:
                            nc.vector.tensor_add(
                                nsp[:, w - 128:w], nsp[:, w - 128:w], triN)
                        nc.scalar.activation(
                            prow[:, cc * 512:cc * 512 + w], nsp[:, 0:w],
                            Exp, scale=0.125)
                    W = (gi + 1) * 128 if causal else S
                    nc.vector.tensor_scalar_mul(
                        prow[:, 0:W], prow[:, 0:W],
                        rns_list[h][:, ii:ii + 1])
                    nc.sync.dma_start(
                        p_out[h, gi * 128:(gi + 1) * 128, 0:W], prow[:, 0:W])
                    if causal and WRITE_ZEROS and gi < NQT - 1:
                        nc.sync.dma_start(
                            p_out[h, gi * 128:(gi + 1) * 128, W:S],
                            zero_t[:, 0:S - W])
            # output projection for this 512-row block
            for ii in range(4):
                gi = 4 * I + ii
                osb = osbp.tile([128, D], f32, tag="osb")
                for nh in range(2):
                    opp = ps_a.tile([128, 512], f32, tag="bank")
                    for h in range(HPC):
                        nc.tensor.matmul(
                            opp,
                            xans[h][:, ii * 128:(ii + 1) * 128],
                            wo_sb[:, h, nh * 512:(nh + 1) * 512],
                            start=(h == 0), stop=(h == HPC - 1))
                    nc.vector.tensor_copy(osb[:, nh * 512:(nh + 1) * 512], opp)
                nc.sync.dma_start(o_part[gi * 128:(gi + 1) * 128, :], osb)

    nc.compile()
    return nc


def _get_nc(causal):
    key = (causal, MM_PRECISE, WRITE_ZEROS)
    if key not in _NC_CACHE:
        _NC_CACHE[key] = _build(causal, MM_PRECISE)
    return _NC_CACHE[key]


_TRIL = None


def _is_causal(mask):
    global _TRIL
    m = np.asarray(mask).reshape(S, S)
    if _TRIL is None:
        _TRIL = np.tril(np.ones((S, S), m.dtype))
    return np.array_equal(m, _TRIL)


def kernel(q, k, v, mask, Wq, bq, Wk, bk, Wv, bv, Wo, bo):
    from concourse import bass_utils
    global LAST_RESULT

    q = np.ascontiguousarray(np.asarray(q, np.float32))
    k = np.ascontiguousarray(np.asarray(k, np.float32))
    v = np.ascontiguousarray(np.asarray(v, np.float32))
    Wq, Wk, Wv, Wo = [np.asarray(t, np.float32) for t in (Wq, Wk, Wv, Wo)]
    bq, bk, bv, bo = [np.asarray(t, np.float32) for t in (bq, bk, bv, bo)]

    causal = _is_causal(mask)
    nc = _get_nc(causal)

    extra = {}
    if not causal:
        m = np.asarray(mask).reshape(S, S)
        mbias = np.where(m == 0, np.float32(-8e9), np.float32(0.0)).astype(np.float32)
        extra = {"mb": np.ascontiguousarray(mbias),
                 "mbT": np.ascontiguousarray(mbias.T)}

    in_maps = []
    for cidx in range(NCORES):
        b_, hg = divmod(cidx, 4)
        sl = slice(hg * HPC * DH, (hg + 1) * HPC * DH)
        in_maps.append({
            "xq": q[b_], "xk": k[b_], "xv": v[b_],
            "wq": np.ascontiguousarray(Wq[:, sl]),
            "wk": np.ascontiguousarray(Wk[:, sl]),
            "wv": np.ascontiguousarray(Wv[:, sl]),
            "bq": np.ascontiguousarray(bq[sl]),
            "bk": np.ascontiguousarray(bk[sl]),
            "bv": np.ascontiguousarray(bv[sl]),
            "wo": np.ascontiguousarray(Wo[sl, :]),
            **extra,
        })

    res = bass_utils.run_bass_kernel_spmd(
        nc, in_maps, core_ids=list(range(NCORES)), trace=TRACE)
    LAST_RESULT = res

    p_attn = np.empty((B, H, S, S), np.float32)
    out = np.zeros((B, S, D), np.float32)
    for cidx in range(NCORES):
        b_, hg = divmod(cidx, 4)
        r = res.results[cidx]
        p_attn[b_, hg * HPC:(hg + 1) * HPC] = r["p_out"]
        out[b_] += r["o_part"]
    out += bo
    return out, p_attn


# revision 12
# speedup vs baseline: 2.3809x; 1.1112x over previous
"""Multi-head attention (QKV proj -> causal attention -> out proj) on 8 TRN2 cores.

Sharding: batch x head-group. Core c handles batch c//4, heads (c%4)*4 .. +4.
Each core:
  - projects its batch's q/k/v rows with its 256-column slice of Wq/Wk/Wv
    (keeping Q^T / K^T with head-dim on partitions, V natural with an extra
    ones-column so the attention-weight row sums fall out of the A@V matmul),
  - computes scores twice (transposed [k,q] for the A@V contraction, natural
    [q,k] for the p_attn output rows), exp via ScalarE without max-subtraction
    (scores are O(6), fp32 exp is safe), causal masking via affine_select,
    softmax normalization folded into the output side,
  - writes its 4 heads' p_attn shard and a partial (row-sharded Wo) output
    projection; host sums the partials and adds bo.

Matmuls run in float32r (TF32-like, 1 cycle/row vs 4 for fp32; measured
~1.6e-4 absmax rel error per matmul on N(0,1) data). Set MM_PRECISE=True
to rebuild with full-fp32 matmuls.

Returns (out, p_attn) matching the reference.
"""
import sys

if "/opt/trn_rl_repo" not in sys.path:
    sys.path.insert(0, "/opt/trn_rl_repo")

import numpy as np

B, S, D = 2, 2048, 1024
H, DH = 16, 64
HPC = 4          # heads per core
NCORES = 8
NQT = S // 128   # 16 query tiles of 128 rows
NI = S // 512    # 4 query super-tiles of 512 rows

TRACE = False
LAST_RESULT = None
WRITE_ZEROS = False
MM_PRECISE = False

_NC_CACHE = {}


def _build(causal, precise):
    import concourse.bacc as bacc
    import concourse.tile as tile
    from concourse import mybir
    from concourse.masks import make_identity
    from contextlib import ExitStack

    f32 = mybir.dt.float32
    mmdt = f32 if precise else mybir.dt.float32r
    Exp = mybir.ActivationFunctionType.Exp
    Ident = mybir.ActivationFunctionType.Identity
    is_ge = mybir.AluOpType.is_ge

    nc = bacc.Bacc("TRN2", target_bir_lowering=False, debug=False, num_devices=NCORES)

    xq = nc.dram_tensor("xq", [S, D], mmdt, kind="ExternalInput").ap()
    xk = nc.dram_tensor("xk", [S, D], mmdt, kind="ExternalInput").ap()
    xv = nc.dram_tensor("xv", [S, D], mmdt, kind="ExternalInput").ap()
    wq = nc.dram_tensor("wq", [D, HPC * DH], mmdt, kind="ExternalInput").ap()
    wk = nc.dram_tensor("wk", [D, HPC * DH], mmdt, kind="ExternalInput").ap()
    wv = nc.dram_tensor("wv", [D, HPC * DH], mmdt, kind="ExternalInput").ap()
    bq = nc.dram_tensor("bq", [HPC * DH], f32, kind="ExternalInput").ap()
    bk = nc.dram_tensor("bk", [HPC * DH], f32, kind="ExternalInput").ap()
    bv = nc.dram_tensor("bv", [HPC * DH], f32, kind="ExternalInput").ap()
    wo = nc.dram_tensor("wo", [HPC * DH, D], mmdt, kind="ExternalInput").ap()
    if not causal:
        mb = nc.dram_tensor("mb", [S, S], f32, kind="ExternalInput").ap()
        mbT = nc.dram_tensor("mbT", [S, S], f32, kind="ExternalInput").ap()
    p_out = nc.dram_tensor("p_out", [HPC, S, S], f32, kind="ExternalOutput").ap()
    o_part = nc.dram_tensor("o_part", [S, D], f32, kind="ExternalOutput").ap()

    with ExitStack() as ctx:
        tc = ctx.enter_context(tile.TileContext(nc))
        persist = ctx.enter_context(tc.tile_pool(name="persist", bufs=1))
        xpool = ctx.enter_context(tc.tile_pool(name="xpool", bufs=3))
        bigp = ctx.enter_context(tc.tile_pool(name="bigp", bufs=6))
        etp = ctx.enter_context(tc.tile_pool(name="etp", bufs=8))
        xanp = ctx.enter_context(tc.tile_pool(name="xanp", bufs=6))
        osbp = ctx.enter_context(tc.tile_pool(name="osbp", bufs=2))
        smallp = ctx.enter_context(tc.tile_pool(name="smallp", bufs=2))
        mbp = None
        if not causal:
            mbp = ctx.enter_context(tc.tile_pool(name="mbp", bufs=3))
        ps_a = ctx.enter_context(tc.tile_pool(name="ps_a", bufs=6, space="PSUM"))
        ps_x = ctx.enter_context(tc.tile_pool(name="ps_x", bufs=2, space="PSUM"))

        ident = persist.tile([128, 128], mmdt)
        identf = persist.tile([128, 128], f32)
        make_identity(nc, identf)
        nc.vector.tensor_copy(ident, identf)

        qT = persist.tile([128, 2, S], mmdt)     # [dh_local, chunk, s]
        kT = persist.tile([128, 2, S], mmdt)
        v_nat = persist.tile([128, NQT, HPC, DH + 1], mmdt)  # [s_loc, s_chunk, h, dh|1]
        onescol = persist.tile([128, 1], f32)
        nc.gpsimd.memset(onescol, 1.0)
        nc.vector.tensor_copy(
            v_nat[:, :, :, DH:DH + 1],
            onescol.to_broadcast([128, NQT, HPC, 1]))

        wq_sb = persist.tile([128, 8, HPC * DH], mmdt)
        wk_sb = persist.tile([128, 8, HPC * DH], mmdt)
        wv_sb = persist.tile([128, 8, HPC * DH], mmdt)
        nc.sync.dma_start(wq_sb, wq.rearrange("(c p) m -> p c m", p=128))
        nc.sync.dma_start(wk_sb, wk.rearrange("(c p) m -> p c m", p=128))
        nc.sync.dma_start(wv_sb, wv.rearrange("(c p) m -> p c m", p=128))
        wo_sb = persist.tile([64, HPC, D], mmdt)  # [dh, h, n]
        nc.sync.dma_start(wo_sb, wo.rearrange("(h p) n -> p h n", p=64))

        bq_sb = persist.tile([128, 2], f32)
        bk_sb = persist.tile([128, 2], f32)
        nc.sync.dma_start(bq_sb, bq.rearrange("(c p) -> p c", p=128))
        nc.sync.dma_start(bk_sb, bk.rearrange("(c p) -> p c", p=128))
        bv_sb = persist.tile([64, HPC], f32)   # [dh, h]
        nc.sync.dma_start(bv_sb, bv.rearrange("(h p) -> p h", p=64))

        ones_f = persist.tile([1, 64], f32)
        nc.gpsimd.memset(ones_f, 1.0)

        # additive causal masks. triG is a sliding window: slicing
        # triG[:, 384-o : 512] (width o+128) masks a transposed-scores diag
        # block whose k-tile starts o columns into the q-supertile.
        # triG[p,f] = -1e9 where (f-384) < p else 0.
        triG = persist.tile([128, 512], f32)
        nc.gpsimd.memset(triG, 0.0)
        nc.gpsimd.affine_select(out=triG, in_=triG, compare_op=is_ge,
                                fill=-1e9, base=-384, channel_multiplier=-1,
                                pattern=[[1, 512]])
        triN = persist.tile([128, 128], f32)
        nc.gpsimd.memset(triN, 0.0)
        nc.gpsimd.affine_select(out=triN, in_=triN, compare_op=is_ge,
                                fill=-1e9, base=0, channel_multiplier=1,
                                pattern=[[-1, 128]])

        zero_t = None
        if causal and WRITE_ZEROS:
            zero_t = persist.tile([128, S - 128], f32)
            nc.gpsimd.memset(zero_t, 0.0)

        # ---------------- phase A: projections ----------------
        for t_idx, (x_in, w_sb) in enumerate([(xq, wq_sb), (xk, wk_sb), (xv, wv_sb)]):
            for st in range(4):      # s-tiles of 512
                slab_a = bigp.tile([128, 4, 512], mmdt, tag="big")
                slab_b = bigp.tile([128, 4, 512], mmdt, tag="big")
                slabs = [slab_a, slab_b]
                slab_of = lambda dc: slabs[dc // 4][:, dc % 4, :]
                for scn in range(4):  # s-chunks of 128
                    xt = xpool.tile([128, D], mmdt, tag="x")
                    s0 = st * 512 + scn * 128
                    nc.sync.dma_start(xt, x_in[s0:s0 + 128, :])
                    for dc in range(8):
                        trp = ps_a.tile([128, 128], mmdt, tag="bank")
                        nc.tensor.transpose(trp, xt[:, dc * 128:(dc + 1) * 128], ident)
                        nc.scalar.activation(
                            slab_of(dc)[:, scn * 128:(scn + 1) * 128], trp, Ident)
                if t_idx < 2:
                    dest, b_sb = (qT, bq_sb) if t_idx == 0 else (kT, bk_sb)
                    for c in range(2):
                        pp = ps_a.tile([128, 512], f32, tag="bank")
                        for dc in range(8):
                            nc.tensor.matmul(
                                pp, w_sb[:, dc, c * 128:(c + 1) * 128],
                                slab_of(dc),
                                start=(dc == 0), stop=(dc == 7))
                        nc.scalar.activation(
                            dest[:, c, st * 512:(st + 1) * 512], pp, Ident,
                            bias=b_sb[:, c:c + 1])
                else:
                    for scn in range(4):
                        sc = st * 4 + scn
                        pp = ps_a.tile([128, 256], f32, tag="bank")
                        for dc in range(8):
                            nc.tensor.matmul(
                                pp, slab_of(dc)[:, scn * 128:(scn + 1) * 128],
                                w_sb[:, dc, :],
                                start=(dc == 0), stop=(dc == 7))
                        nc.vector.tensor_copy(
                            v_nat[:, sc, :, 0:DH],
                            pp.rearrange("p (h d) -> p h d", h=HPC))

        # ---------------- phase B: attention ----------------
        for I in range(NI):
            xans = []          # per head [64, 512] normalized x_att^T (+bv)
            rns_list = []      # per head [128, 4] natural-layout recip
            for h in range(HPC):
                c, po = h // 2, (h % 2) * 64
                nj = 4 * I + 4 if causal else NQT
                xatt = ps_x.tile([DH + 1, 512], f32, tag="xatt")
                for j in range(nj):
                    scp = ps_a.tile([128, 512], f32, tag="bank")
                    nc.tensor.matmul(
                        scp,
                        kT[po:po + 64, c, j * 128:(j + 1) * 128],
                        qT[po:po + 64, c, I * 512:(I + 1) * 512],
                        start=True, stop=True)
                    if not causal:
                        mt = mbp.tile([128, 512], f32, tag="mbT")
                        nc.sync.dma_start(
                            mt, mbT[j * 128:(j + 1) * 128, I * 512:(I + 1) * 512])
                        nc.vector.tensor_add(scp, scp, mt)
                    if causal and j >= 4 * I:
                        o = (j - 4 * I) * 128
                        nc.vector.tensor_add(
                            scp[:, 0:o + 128], scp[:, 0:o + 128],
                            triG[:, 384 - o:512])
                    eT = etp.tile([128, 512], mmdt, tag="eT")
                    nc.scalar.activation(eT, scp, Exp, scale=0.125)
                    nc.tensor.matmul(
                        xatt, v_nat[:, j, h, :], eT,
                        start=(j == 0), stop=(j == nj - 1))
                # normalizers: bcs = 1/sums broadcast over 64 partitions,
                # rns = 1/sums in natural [q,1] layout
                sums = smallp.tile([1, 512], f32, tag="sums")
                nc.vector.tensor_copy(sums, xatt[DH:DH + 1, :])
                bcp = ps_a.tile([64, 512], f32, tag="bank")
                nc.tensor.matmul(bcp, ones_f, sums, start=True, stop=True)
                bcs = smallp.tile([64, 512], f32, tag="bcs")
                nc.vector.reciprocal(bcs, bcp)
                xan = xanp.tile([64, 512], mmdt, tag="xan")
                nc.vector.tensor_mul(xan, xatt[0:DH, :], bcs)
                nc.vector.tensor_scalar_add(xan, xan, bv_sb[:, h:h + 1])
                xans.append(xan)
                # natural-layout recip column per 128-query subtile
                rnp = ps_a.tile([128, 4], f32, tag="bank")
                for ii in range(4):
                    nc.tensor.matmul(
                        rnp[:, ii:ii + 1],
                        sums[0:1, ii * 128:(ii + 1) * 128],
                        ones_f[0:1, 0:1],
                        start=True, stop=True)
                rns = smallp.tile([128, 4], f32, tag="rns")
                nc.vector.reciprocal(rns, rnp)
                rns_list.append(rns)
            # natural side: p_attn rows
            for h in range(HPC):
                c, po = h // 2, (h % 2) * 64
                for ii in range(4):
                    gi = 4 * I + ii
                    nch = (gi // 4 + 1) if causal else 4
                    wlast = (gi % 4) * 128 + 128
                    prow = bigp.tile([128, S], f32, tag="big")
                    for cc in range(nch):
                        w = 512 if (not causal or cc < gi // 4) else wlast
                        nsp = ps_a.tile([128, 512], f32, tag="bank")
                        nc.tensor.matmul(
                            nsp[:, 0:w],
                            qT[po:po + 64, c, gi * 128:(gi + 1) * 128],
                            kT[po:po + 64, c, cc * 512:cc * 512 + w],
                            start=True, stop=True)
                        if not causal:
                            mt2 = mbp.tile([128, 512], f32, tag="mb")
                            nc.sync.dma_start(
                                mt2, mb[gi * 128:(gi + 1) * 128,
                                        cc * 512:(cc + 1) * 512])
                            nc.vector.tensor_add(nsp, nsp, mt2)
                        if causal and cc == nch - 1:
                            nc.vector.tensor_add(
                                nsp[:, w - 128:w], nsp[:, w - 128:w], triN)
                        nc.scalar.activation(
                            prow[:, cc * 512:cc * 512 + w], nsp[:, 0:w],
                            Exp, scale=0.125)
                    W = (gi + 1) * 128 if causal else S
                    nc.vector.tensor_scalar_mul(
                        prow[:, 0:W], prow[:, 0:W],
                        rns_list[h][:, ii:ii + 1])
                    nc.sync.dma_start(
                        p_out[h, gi * 128:(gi + 1) * 128, 0:W], prow[:, 0:W])
                    if causal and WRITE_ZEROS and gi < NQT - 1:
                        nc.sync.dma_start(
                            p_out[h, gi * 128:(gi + 1) * 128, W:S],
                            zero_t[:, 0:S - W])
            # output projection for this 512-row block
            for ii in range(4):
                gi = 4 * I + ii
                osb = osbp.tile([128, D], f32, tag="osb")
                for nh in range(2):
                    opp = ps_a.tile([128, 512], f32, tag="bank")
                    for h in range(HPC):
                        nc.tensor.matmul(
                            opp,
                            xans[h][:, ii * 128:(ii + 1) * 128],
                            wo_sb[:, h, nh * 512:(nh + 1) * 512],
                            start=(h == 0), stop=(h == HPC - 1))
                    nc.vector.tensor_copy(osb[:, nh * 512:(nh + 1) * 512], opp)
                nc.sync.dma_start(o_part[gi * 128:(gi + 1) * 128, :], osb)

    nc.compile()
    return nc


def _get_nc(causal):
    key = (causal, MM_PRECISE, WRITE_ZEROS)
    if key not in _NC_CACHE:
        _NC_CACHE[key] = _build(causal, MM_PRECISE)
    return _NC_CACHE[key]


_TRIL = None


def _is_causal(mask):
    global _TRIL
    m = np.asarray(mask).reshape(S, S)
    if _TRIL is None:
        _TRIL = np.tril(np.ones((S, S), m.dtype))
    return np.array_equal(m, _TRIL)


def kernel(q, k, v, mask, Wq, bq, Wk, bk, Wv, bv, Wo, bo):
    from concourse import bass_utils
    global LAST_RESULT

    q = np.ascontiguousarray(np.asarray(q, np.float32))
    k = np.ascontiguousarray(np.asarray(k, np.float32))
    v = np.ascontiguousarray(np.asarray(v, np.float32))
    Wq, Wk, Wv, Wo = [np.asarray(t, np.float32) for t in (Wq, Wk, Wv, Wo)]
    bq, bk, bv, bo = [np.asarray(t, np.float32) for t in (bq, bk, bv, bo)]

    causal = _is_causal(mask)
    nc = _get_nc(causal)

    extra = {}
    if not causal:
        m = np.asarray(mask).reshape(S, S)
        mbias = np.where(m == 0, np.float32(-8e9), np.float32(0.0)).astype(np.float32)
        extra = {"mb": np.ascontiguousarray(mbias),
                 "mbT": np.ascontiguousarray(mbias.T)}

    in_maps = []
    for cidx in range(NCORES):
        b_, hg = divmod(cidx, 4)
        sl = slice(hg * HPC * DH, (hg + 1) * HPC * DH)
        in_maps.append({
            "xq": q[b_], "xk": k[b_], "xv": v[b_],
            "wq": np.ascontiguousarray(Wq[:, sl]),
            "wk": np.ascontiguousarray(Wk[:, sl]),
            "wv": np.ascontiguousarray(Wv[:, sl]),
            "bq": np.ascontiguousarray(bq[sl]),
            "bk": np.ascontiguousarray(bk[sl]),
            "bv": np.ascontiguousarray(bv[sl]),
            "wo": np.ascontiguousarray(Wo[sl, :]),
            **extra,
        })

    res = bass_utils.run_bass_kernel_spmd(
        nc, in_maps, core_ids=list(range(NCORES)), trace=TRACE)
    LAST_RESULT = res

    p_attn = np.empty((B, H, S, S), np.float32)
    out = np.zeros((B, S, D), np.float32)
    for cidx in range(NCORES):
        b_, hg = divmod(cidx, 4)
        r = res.results[cidx]
        p_attn[b_, hg * HPC:(hg + 1) * HPC] = r["p_out"]
        out[b_] += r["o_part"]
    out += bo
    return out, p_attn
